# revision 10
# baseline (speedup 1.0000x reference)
"""DOMINO++ loss kernel for Trainium2 (8 NeuronCores, data-parallel).

Strategy
--------
Shard the (n=2, c=12, 96^3) logits over 8 cores: 4 contiguous spatial
blocks per batch element (cores 0-3 -> n=0, cores 4-7 -> n=1).  Inputs
ship as bf16 (halves DMA; the per-element rounding noise statistically
cancels in the ~2e4..2e5-element sums this kernel reduces to).
Each core reduces its 221184 voxels to a small set of partial sums:

  - A [96, 192] PSUM accumulated on the TensorEngine holding 8 diagonal
    [12, 24] blocks (JB=8 voxel-columns batched per matmul; host sums
    the blocks):
      lhsT (stationary) = onehot masks m_t (12 planes x 8 columns)
      rhs  (moving)     = probs g_c | raw logits x_c
      M[t, c]    = sum_v m_t(v) g_c(v)   dice: inter = diag, ground =
                                         row sums, pred = col sums (masks
                                         are a partition of unity);
                                         penalty = <P, M> Frobenius
      X[t, t]    = sum_{v in t} x_t(v)   CE logit-gather term
  - logdacc[128, NCH] f32: row sums of log(sum_c exp x_c) (ACT accum_out)

Host combines the 8 cores' tiny outputs into the scalar loss.

Engine split per chunk (tiles [128, F] voxel-major, all bulk data bf16):
  DMA: 12 full-height class rows, contiguous 3456B runs, spread over the
       sync/scalar HWDGE + gpsimd SWDGE queues
  DVE: interleave-copy of x for PE; 12 onehot masks; 4 merged tree-adds
       for the softmax denominator; 12 prob muls
  ACT: 12 exp (in-place), ln(denom)+accum, recip = exp(-ln d)
  PE:  2 matmuls per 8-column group (probs block, x block)
"""

import os
import sys
from contextlib import ExitStack

import numpy as np

sys.path.insert(0, "/opt/trn_rl_repo")

from concourse import bacc, bass, mybir, tile  # noqa: E402
from concourse import bass_utils  # noqa: E402

F32 = mybir.dt.float32
BF16 = mybir.dt.bfloat16
ALU = mybir.AluOpType
ACTF = mybir.ActivationFunctionType

N_CORES = 8
C = 12            # classes
P = 128           # SBUF partitions
FT = 1728         # free size per partition per core (P*FT = 221184 voxels)
NCH = 4           # chunks
FC = FT // NCH    # free columns per chunk (432)
JB = 8            # voxel-columns batched per matmul (12*JB <= 128)
G = FC // JB      # matmul groups per chunk (54)
S = P * FT        # voxels per core
N, H, W, Z = 2, 96, 96, 96
SPATIAL = H * W * Z          # 884736 voxels per batch element
CORES_PER_N = N_CORES // N   # 4

_CACHE = {}


def _build_program():
    """Build + compile the per-core Bass program (identical on all cores)."""
    nc = bacc.Bacc("TRN2", target_bir_lowering=False, debug=False,
                   num_devices=N_CORES)

    x_d = nc.dram_tensor("x", (C, S), BF16, kind="ExternalInput")
    t_d = nc.dram_tensor("t", (S,), BF16, kind="ExternalInput")
    cls_d = nc.dram_tensor("cls", (C,), BF16, kind="ExternalInput")
    m_d = nc.dram_tensor("m_out", (12 * JB, 24 * JB), F32,
                         kind="ExternalOutput")
    logd_d = nc.dram_tensor("logd_out", (P, NCH), F32, kind="ExternalOutput")

    x_v = x_d.rearrange("c (p f) -> c p f", p=P)
    t_v = t_d.rearrange("(p f) -> p f", p=P)

    with ExitStack() as ctx:
        tc = ctx.enter_context(tile.TileContext(nc))
        sb = ctx.enter_context(tc.tile_pool(name="sb", bufs=2))
        acc = ctx.enter_context(tc.tile_pool(name="acc", bufs=1))
        ps = ctx.enter_context(tc.tile_pool(name="ps", bufs=1, space="PSUM"))

        logdacc = acc.tile([P, NCH], F32)
        psum_m = ps.tile([12 * JB, 24 * JB], F32)

        # full-height input tiles: 13 big DMAs with 3456B contiguous runs,
        # spread across the three available DGE queues
        xt = acc.tile([P, C, FT], BF16)
        tt = acc.tile([P, FT], BF16)
        cls = acc.tile([P, C], BF16)     # 0..11, replicated per partition
        for c in range(C):
            eng = (nc.sync, nc.scalar, nc.gpsimd)[c % 3]
            eng.dma_start(xt[:, c], x_v[c])
        nc.sync.dma_start(tt[:], t_v[:])
        nc.sync.dma_start(cls[:], cls_d[:].partition_broadcast(P))

        chunk_state = {}

        def phase_exp(ch):
            sl = slice(ch * FC, (ch + 1) * FC)
            gxi = sb.tile([P, G, 2 * C, JB], BF16, tag="gxi", name=f"gxi{ch}")
            mk = sb.tile([P, G, C, JB], BF16, tag="mk", name=f"mk{ch}")
            tmp6 = sb.tile([P, 6, FC], BF16, tag="tmp6", name=f"t6_{ch}")
            tmp3 = sb.tile([P, 3, FC], BF16, tag="tmp3", name=f"t3_{ch}")
            dna = sb.tile([P, FC], BF16, tag="dna", name=f"dna{ch}")
            dn = sb.tile([P, FC], BF16, tag="dn", name=f"dn{ch}")
            lg = sb.tile([P, FC], F32, tag="lg", name=f"lg{ch}")
            rc = sb.tile([P, FC], BF16, tag="rc", name=f"rc{ch}")
            chunk_state[ch] = (sl, gxi, mk, tmp6, tmp3, dna, dn, lg, rc)

            # interleaved copy of the raw logits for the PE x-block
            nc.vector.tensor_copy(
                gxi[:, :, C:].rearrange("p g c j -> p c g j"),
                xt[:, :, sl].rearrange("p c (g j) -> p c g j", j=JB))

            # onehot masks, all classes in one op via broadcast operands
            tg_b = tt[:, sl].rearrange("p (g j) -> p () g j", j=JB) \
                .to_broadcast([P, C, G, JB])
            cls_b = cls[:].rearrange("p c -> p c () ()") \
                .to_broadcast([P, C, G, JB])
            nc.vector.tensor_tensor(mk[:].rearrange("p g c j -> p c g j"),
                                    tg_b, cls_b, op=ALU.is_equal)

            # exp in-place, one op (after the xi snapshot; WAR ordered)
            nc.scalar.activation(xt[:, :, sl], xt[:, :, sl], ACTF.Exp)

            # softmax denominator: merged pairwise tree adds
            nc.vector.tensor_tensor(tmp6[:], xt[:, 0::2, sl], xt[:, 1::2, sl],
                                    op=ALU.add)
            nc.vector.tensor_tensor(tmp3[:], tmp6[:, 0::2], tmp6[:, 1::2],
                                    op=ALU.add)
            nc.vector.tensor_tensor(dna[:], tmp3[:, 0], tmp3[:, 1],
                                    op=ALU.add)
            nc.vector.tensor_tensor(dn[:], dna[:], tmp3[:, 2], op=ALU.add)

        def phase_ln(ch):
            (sl, gxi, mk, tmp6, tmp3, dna, dn, lg, rc) = chunk_state[ch]
            nc.scalar.activation(lg[:], dn[:], ACTF.Ln,
                                 accum_out=logdacc[:, ch:ch + 1])

        def phase_tail(ch):
            (sl, gxi, mk, tmp6, tmp3, dna, dn, lg, rc) = chunk_state[ch]
            nc.scalar.activation(rc[:], lg[:], ACTF.Exp, scale=-1.0)

            # probs g_c = exp_c * recip, one op via broadcast recip
            rc_b = rc[:].rearrange("p (g j) -> p () g j", j=JB) \
                .to_broadcast([P, C, G, JB])
            nc.vector.tensor_tensor(
                gxi[:, :, :C].rearrange("p g c j -> p c g j"),
                xt[:, :, sl].rearrange("p c (g j) -> p c g j", j=JB),
                rc_b, op=ALU.mult)

            # statistics matrix on PE: one [128,96]x[128,192] matmul/group
            for g in range(G):
                nc.tensor.matmul(psum_m[:], mk[:, g], gxi[:, g],
                                 start=(ch == 0 and g == 0),
                                 stop=(ch == NCH - 1 and g == G - 1))

        # chunk pairs share one Ln table phase to limit ACT table swaps
        for ch0 in range(0, NCH, 2):
            phase_exp(ch0)
            phase_exp(ch0 + 1)
            phase_ln(ch0)
            phase_ln(ch0 + 1)
            phase_tail(ch0)
            phase_tail(ch0 + 1)

        m_sb = acc.tile([12 * JB, 24 * JB], F32)
        nc.vector.tensor_copy(m_sb[:], psum_m[:])
        nc.sync.dma_start(m_d[:], m_sb[:])
        nc.sync.dma_start(logd_d[:], logdacc[:])

    nc.compile()
    return nc


def _get_program():
    if "nc" not in _CACHE:
        _CACHE["nc"] = _build_program()
    return _CACHE["nc"]


def _shard_inputs(input, target):
    """Full (2,12,96,96,96)/(2,1,96,96,96) -> 8 per-core in_maps (bf16)."""
    bf16 = mybir.dt.np(BF16)
    x = np.asarray(input, dtype=np.float32)
    tg = np.asarray(target).reshape(N, SPATIAL)
    in_maps = []
    for k in range(N_CORES):
        n = k // CORES_PER_N
        o = (k % CORES_PER_N) * S
        xs = np.ascontiguousarray(
            x[n].reshape(C, SPATIAL)[:, o:o + S]).astype(bf16)
        ts = np.ascontiguousarray(tg[n, o:o + S]).astype(np.float32) \
            .astype(bf16)
        in_maps.append({"x": xs, "t": ts,
                        "cls": np.arange(C, dtype=np.float32).astype(bf16)})
    return in_maps


def _combine(results, matrix_penalty, global_step, maxiter):
    pen = np.asarray(matrix_penalty, dtype=np.float64)
    inter = np.zeros((N, C))
    ground = np.zeros((N, C))
    pred = np.zeros((N, C))
    xtgt_sum = 0.0
    logd_sum = 0.0
    pen_sum = 0.0
    for k, r in enumerate(results):
        n = k // CORES_PER_N
        mfull = np.asarray(r["m_out"], dtype=np.float64) \
            .reshape(C, JB, 2 * C, JB)
        m = np.einsum("tjcj->tc", mfull)        # sum the JB diagonal blocks
        mg = m[:, :C]                           # sum_v m_t * g_c
        inter[n] += np.diag(mg)
        ground[n] += mg.sum(axis=1)
        pred[n] += mg.sum(axis=0)               # masks partition unity
        xtgt_sum += np.trace(m[:, C:2 * C])
        logd_sum += float(np.asarray(r["logd_out"], dtype=np.float64).sum())
        pen_sum += float((pen * mg).sum())

    nvox = N * SPATIAL
    dice = 1.0 - (2.0 * inter + 1e-5) / (ground + pred + 1e-5)
    dice_loss = dice.mean()
    ce = (logd_sum - xtgt_sum) / nvox
    ce_total = dice_loss + ce
    pen_mean = pen_sum / nvox
    beta = 10.0 ** np.floor(np.log10(ce_total))
    gs = float(global_step)
    mi = float(maxiter)
    alpha0 = 1.0 - gs / mi
    alpha1 = gs / mi
    return np.float32(alpha1 * ce_total + alpha0 * beta * pen_mean)


def kernel(input, target, matrix_penalty, global_step, maxiter):
    nc = _get_program()
    in_maps = _shard_inputs(input, target)
    trace = bool(int(os.environ.get("BASS_LOSS_TRACE", "0")))
    res = bass_utils.run_bass_kernel_spmd(
        nc, in_maps, core_ids=list(range(N_CORES)), trace=trace)
    _CACHE["last_exec_ns"] = res.exec_time_ns
    return _combine(res.results, matrix_penalty, global_step, maxiter)


# revision 12
# speedup vs baseline: 1.0693x; 1.0693x over previous
"""DOMINO++ loss kernel for Trainium2 (8 NeuronCores, data-parallel).

Strategy
--------
Shard the (n=2, c=12, 96^3) logits over 8 cores: 4 contiguous spatial
blocks per batch element (cores 0-3 -> n=0, cores 4-7 -> n=1).  Inputs
ship as bf16 (halves DMA; the per-element rounding noise statistically
cancels in the ~2e4..2e5-element sums this kernel reduces to).
Each core reduces its 221184 voxels to a small set of partial sums:

  - A [96, 192] PSUM accumulated on the TensorEngine holding 8 diagonal
    [12, 24] blocks (JB=8 voxel-columns batched per matmul; host sums
    the blocks):
      lhsT (stationary) = onehot masks m_t (12 planes x 8 columns)
      rhs  (moving)     = probs g_c | raw logits x_c
      M[t, c]    = sum_v m_t(v) g_c(v)   dice: inter = diag, ground =
                                         row sums, pred = col sums (masks
                                         are a partition of unity);
                                         penalty = <P, M> Frobenius
      X[t, t]    = sum_{v in t} x_t(v)   CE logit-gather term
  - logdacc[128, NCH] f32: row sums of log(sum_c exp x_c) (ACT accum_out)

Host combines the 8 cores' tiny outputs into the scalar loss.

Engine split per chunk (tiles [128, F] voxel-major, all bulk data bf16):
  DMA: 12 full-height class rows, contiguous 3456B runs, spread over the
       sync/scalar HWDGE + gpsimd SWDGE queues
  DVE: interleave-copy of x for PE; 12 onehot masks; 4 merged tree-adds
       for the softmax denominator; 12 prob muls
  ACT: 12 exp (in-place), ln(denom)+accum, recip = exp(-ln d)
  PE:  2 matmuls per 8-column group (probs block, x block)
"""

import os
import sys
from contextlib import ExitStack

import numpy as np

sys.path.insert(0, "/opt/trn_rl_repo")

from concourse import bacc, bass, mybir, tile  # noqa: E402
from concourse import bass_utils  # noqa: E402

F32 = mybir.dt.float32
BF16 = mybir.dt.bfloat16
ALU = mybir.AluOpType
ACTF = mybir.ActivationFunctionType

N_CORES = 8
C = 12            # classes
P = 128           # SBUF partitions
FT = 1728         # free size per partition per core (P*FT = 221184 voxels)
NCH = 4           # chunks
FC = FT // NCH    # free columns per chunk (432)
JB = 8            # voxel-columns batched per matmul (12*JB <= 128)
G = FC // JB      # matmul groups per chunk (54)
S = P * FT        # voxels per core
N, H, W, Z = 2, 96, 96, 96
SPATIAL = H * W * Z          # 884736 voxels per batch element
CORES_PER_N = N_CORES // N   # 4

_CACHE = {}


def _build_program():
    """Build + compile the per-core Bass program (identical on all cores)."""
    nc = bacc.Bacc("TRN2", target_bir_lowering=False, debug=False,
                   num_devices=N_CORES)

    x_d = nc.dram_tensor("x", (C, S), BF16, kind="ExternalInput")
    t_d = nc.dram_tensor("t", (S,), BF16, kind="ExternalInput")
    cls_d = nc.dram_tensor("cls", (C,), BF16, kind="ExternalInput")
    m_d = nc.dram_tensor("m_out", (12 * JB, 24 * JB), F32,
                         kind="ExternalOutput")
    logd_d = nc.dram_tensor("logd_out", (P, NCH), F32, kind="ExternalOutput")

    x_v = x_d.rearrange("c (p f) -> c p f", p=P)
    t_v = t_d.rearrange("(p f) -> p f", p=P)

    with ExitStack() as ctx:
        tc = ctx.enter_context(tile.TileContext(nc))
        sb = ctx.enter_context(tc.tile_pool(name="sb", bufs=3))
        acc = ctx.enter_context(tc.tile_pool(name="acc", bufs=1))
        ps = ctx.enter_context(tc.tile_pool(name="ps", bufs=1, space="PSUM"))

        logdacc = acc.tile([P, NCH], F32)
        psum_m = ps.tile([12 * JB, 24 * JB], F32)

        # full-height input tiles: 13 big DMAs with 3456B contiguous runs,
        # spread across the three available DGE queues
        xt = acc.tile([P, C, FT], BF16)
        tt = acc.tile([P, FT], BF16)
        cls = acc.tile([P, C], BF16)     # 0..11, replicated per partition
        for c in range(C):
            eng = (nc.sync, nc.scalar, nc.gpsimd)[c % 3]
            eng.dma_start(xt[:, c], x_v[c])
        nc.sync.dma_start(tt[:], t_v[:])
        nc.sync.dma_start(cls[:], cls_d[:].partition_broadcast(P))

        chunk_state = {}

        def phase_exp(ch):
            sl = slice(ch * FC, (ch + 1) * FC)
            gxi = sb.tile([P, G, 2 * C, JB], BF16, tag="gxi", name=f"gxi{ch}")
            mk = sb.tile([P, G, C, JB], BF16, tag="mk", name=f"mk{ch}")
            tmp6 = sb.tile([P, 6, FC], BF16, tag="tmp6", name=f"t6_{ch}")
            tmp3 = sb.tile([P, 3, FC], BF16, tag="tmp3", name=f"t3_{ch}")
            dna = sb.tile([P, FC], BF16, tag="dna", name=f"dna{ch}")
            dn = sb.tile([P, FC], BF16, tag="dn", name=f"dn{ch}")
            lg = sb.tile([P, FC], F32, tag="lg", name=f"lg{ch}")
            rc = sb.tile([P, FC], BF16, tag="rc", name=f"rc{ch}")
            chunk_state[ch] = (sl, gxi, mk, tmp6, tmp3, dna, dn, lg, rc)

            # interleaved copy of the raw logits for the PE x-block
            nc.vector.tensor_copy(
                gxi[:, :, C:].rearrange("p g c j -> p c g j"),
                xt[:, :, sl].rearrange("p c (g j) -> p c g j", j=JB))

            # onehot masks, all classes in one op via broadcast operands
            tg_b = tt[:, sl].rearrange("p (g j) -> p () g j", j=JB) \
                .to_broadcast([P, C, G, JB])
            cls_b = cls[:].rearrange("p c -> p c () ()") \
                .to_broadcast([P, C, G, JB])
            nc.vector.tensor_tensor(mk[:].rearrange("p g c j -> p c g j"),
                                    tg_b, cls_b, op=ALU.is_equal)

            # exp in-place, one op (after the xi snapshot; WAR ordered)
            nc.scalar.activation(xt[:, :, sl], xt[:, :, sl], ACTF.Exp)

            # softmax denominator: merged pairwise tree adds
            nc.vector.tensor_tensor(tmp6[:], xt[:, 0::2, sl], xt[:, 1::2, sl],
                                    op=ALU.add)
            nc.vector.tensor_tensor(tmp3[:], tmp6[:, 0::2], tmp6[:, 1::2],
                                    op=ALU.add)
            nc.vector.tensor_tensor(dna[:], tmp3[:, 0], tmp3[:, 1],
                                    op=ALU.add)
            nc.vector.tensor_tensor(dn[:], dna[:], tmp3[:, 2], op=ALU.add)

        def phase_ln(ch):
            (sl, gxi, mk, tmp6, tmp3, dna, dn, lg, rc) = chunk_state[ch]
            nc.scalar.activation(lg[:], dn[:], ACTF.Ln,
                                 accum_out=logdacc[:, ch:ch + 1])

        def phase_tail(ch):
            (sl, gxi, mk, tmp6, tmp3, dna, dn, lg, rc) = chunk_state[ch]
            nc.scalar.activation(rc[:], lg[:], ACTF.Exp, scale=-1.0)

            # probs g_c = exp_c * recip, one op via broadcast recip
            rc_b = rc[:].rearrange("p (g j) -> p () g j", j=JB) \
                .to_broadcast([P, C, G, JB])
            nc.vector.tensor_tensor(
                gxi[:, :, :C].rearrange("p g c j -> p c g j"),
                xt[:, :, sl].rearrange("p c (g j) -> p c g j", j=JB),
                rc_b, op=ALU.mult)

            # statistics matrix on PE: one [128,96]x[128,192] matmul/group
            for g in range(G):
                nc.tensor.matmul(psum_m[:], mk[:, g], gxi[:, g],
                                 start=(ch == 0 and g == 0),
                                 stop=(ch == NCH - 1 and g == G - 1))

        for ch in range(NCH):
            phase_exp(ch)
            phase_ln(ch)
            phase_tail(ch)

        m_sb = acc.tile([12 * JB, 24 * JB], F32)
        nc.vector.tensor_copy(m_sb[:], psum_m[:])
        nc.sync.dma_start(m_d[:], m_sb[:])
        nc.sync.dma_start(logd_d[:], logdacc[:])

    nc.compile()
    return nc


def _get_program():
    if "nc" not in _CACHE:
        _CACHE["nc"] = _build_program()
    return _CACHE["nc"]


def _shard_inputs(input, target):
    """Full (2,12,96,96,96)/(2,1,96,96,96) -> 8 per-core in_maps (bf16)."""
    bf16 = mybir.dt.np(BF16)
    x = np.asarray(input, dtype=np.float32)
    tg = np.asarray(target).reshape(N, SPATIAL)
    in_maps = []
    for k in range(N_CORES):
        n = k // CORES_PER_N
        o = (k % CORES_PER_N) * S
        xs = np.ascontiguousarray(
            x[n].reshape(C, SPATIAL)[:, o:o + S]).astype(bf16)
        ts = np.ascontiguousarray(tg[n, o:o + S]).astype(np.float32) \
            .astype(bf16)
        in_maps.append({"x": xs, "t": ts,
                        "cls": np.arange(C, dtype=np.float32).astype(bf16)})
    return in_maps


def _combine(results, matrix_penalty, global_step, maxiter):
    pen = np.asarray(matrix_penalty, dtype=np.float64)
    inter = np.zeros((N, C))
    ground = np.zeros((N, C))
    pred = np.zeros((N, C))
    xtgt_sum = 0.0
    logd_sum = 0.0
    pen_sum = 0.0
    for k, r in enumerate(results):
        n = k // CORES_PER_N
        mfull = np.asarray(r["m_out"], dtype=np.float64) \
            .reshape(C, JB, 2 * C, JB)
        m = np.einsum("tjcj->tc", mfull)        # sum the JB diagonal blocks
        mg = m[:, :C]                           # sum_v m_t * g_c
        inter[n] += np.diag(mg)
        ground[n] += mg.sum(axis=1)
        pred[n] += mg.sum(axis=0)               # masks partition unity
        xtgt_sum += np.trace(m[:, C:2 * C])
        logd_sum += float(np.asarray(r["logd_out"], dtype=np.float64).sum())
        pen_sum += float((pen * mg).sum())

    nvox = N * SPATIAL
    dice = 1.0 - (2.0 * inter + 1e-5) / (ground + pred + 1e-5)
    dice_loss = dice.mean()
    ce = (logd_sum - xtgt_sum) / nvox
    ce_total = dice_loss + ce
    pen_mean = pen_sum / nvox
    beta = 10.0 ** np.floor(np.log10(ce_total))
    gs = float(global_step)
    mi = float(maxiter)
    alpha0 = 1.0 - gs / mi
    alpha1 = gs / mi
    return np.float32(alpha1 * ce_total + alpha0 * beta * pen_mean)


def kernel(input, target, matrix_penalty, global_step, maxiter):
    nc = _get_program()
    in_maps = _shard_inputs(input, target)
    trace = bool(int(os.environ.get("BASS_LOSS_TRACE", "0")))
    res = bass_utils.run_bass_kernel_spmd(
        nc, in_maps, core_ids=list(range(N_CORES)), trace=trace)
    _CACHE["last_exec_ns"] = res.exec_time_ns
    return _combine(res.results, matrix_penalty, global_step, maxiter)


# revision 14
# speedup vs baseline: 1.1531x; 1.0784x over previous
"""DOMINO++ loss kernel for Trainium2 (8 NeuronCores, data-parallel).

Strategy
--------
Shard the (n=2, c=12, 96^3) logits over 8 cores: 4 contiguous spatial
blocks per batch element (cores 0-3 -> n=0, cores 4-7 -> n=1).  Inputs
ship as bf16 (halves DMA; the per-element rounding noise statistically
cancels in the ~2e4..2e5-element sums this kernel reduces to).
Each core reduces its 221184 voxels to a small set of partial sums:

  - A [96, 192] PSUM accumulated on the TensorEngine holding 8 diagonal
    [12, 24] blocks (JB=8 voxel-columns batched per matmul; host sums
    the blocks):
      lhsT (stationary) = onehot masks m_t (12 planes x 8 columns)
      rhs  (moving)     = probs g_c | raw logits x_c
      M[t, c]    = sum_v m_t(v) g_c(v)   dice: inter = diag, ground =
                                         row sums, pred = col sums (masks
                                         are a partition of unity);
                                         penalty = <P, M> Frobenius
      X[t, t]    = sum_{v in t} x_t(v)   CE logit-gather term
  - logdacc[128, NCH] f32: row sums of log(sum_c exp x_c) (ACT accum_out)

Host combines the 8 cores' tiny outputs into the scalar loss.

Engine split per chunk (tiles [128, F] voxel-major, all bulk data bf16):
  DMA: 12 full-height class rows, contiguous 3456B runs, spread over the
       sync/scalar HWDGE + gpsimd SWDGE queues
  DVE: interleave-copy of x for PE; 12 onehot masks; 4 merged tree-adds
       for the softmax denominator; 12 prob muls
  ACT: 12 exp (in-place), ln(denom)+accum, recip = exp(-ln d)
  PE:  2 matmuls per 8-column group (probs block, x block)
"""

import os
import sys
from contextlib import ExitStack

import numpy as np

sys.path.insert(0, "/opt/trn_rl_repo")

from concourse import bacc, bass, mybir, tile  # noqa: E402
from concourse import bass_utils  # noqa: E402

F32 = mybir.dt.float32
BF16 = mybir.dt.bfloat16
ALU = mybir.AluOpType
ACTF = mybir.ActivationFunctionType

N_CORES = 8
C = 12            # classes
P = 128           # SBUF partitions
FT = 1728         # free size per partition per core (P*FT = 221184 voxels)
NCH = 4           # chunks
FC = FT // NCH    # free columns per chunk (432)
JB = 8            # voxel-columns batched per matmul (12*JB <= 128)
G = FC // JB      # matmul groups per chunk (54)
S = P * FT        # voxels per core
N, H, W, Z = 2, 96, 96, 96
SPATIAL = H * W * Z          # 884736 voxels per batch element
CORES_PER_N = N_CORES // N   # 4

_CACHE = {}


def _build_program():
    """Build + compile the per-core Bass program (identical on all cores)."""
    nc = bacc.Bacc("TRN2", target_bir_lowering=False, debug=False,
                   num_devices=N_CORES)

    x_d = nc.dram_tensor("x", (C, S), BF16, kind="ExternalInput")
    t_d = nc.dram_tensor("t", (S,), BF16, kind="ExternalInput")
    cls_d = nc.dram_tensor("cls", (C,), BF16, kind="ExternalInput")
    m_d = nc.dram_tensor("m_out", (12 * JB, 24 * JB), F32,
                         kind="ExternalOutput")
    logd_d = nc.dram_tensor("logd_out", (P, NCH), F32, kind="ExternalOutput")

    x_v = x_d.rearrange("c (ch p f) -> c ch p f", ch=NCH, p=P)
    t_v = t_d.rearrange("(ch p f) -> ch p f", ch=NCH, p=P)

    with ExitStack() as ctx:
        tc = ctx.enter_context(tile.TileContext(nc))
        sb = ctx.enter_context(tc.tile_pool(name="sb", bufs=3))
        acc = ctx.enter_context(tc.tile_pool(name="acc", bufs=1))
        ps = ctx.enter_context(tc.tile_pool(name="ps", bufs=1, space="PSUM"))

        logdacc = acc.tile([P, NCH], F32)
        psum_m = ps.tile([12 * JB, 24 * JB], F32)

        cls = acc.tile([P, C], BF16)     # 0..11, replicated per partition
        nc.sync.dma_start(cls[:], cls_d[:].partition_broadcast(P))

        chunk_state = {}

        def phase_exp(ch):
            xt = sb.tile([P, C, FC], BF16, tag="xt", name=f"xt{ch}")
            tt = sb.tile([P, FC], BF16, tag="tt", name=f"tt{ch}")
            gxi = sb.tile([P, G, 2 * C, JB], BF16, tag="gxi", name=f"gxi{ch}")
            mk = sb.tile([P, G, C, JB], BF16, tag="mk", name=f"mk{ch}")
            tmp6 = sb.tile([P, 6, FC], BF16, tag="tmp6", name=f"t6_{ch}")
            tmp3 = sb.tile([P, 3, FC], BF16, tag="tmp3", name=f"t3_{ch}")
            dna = sb.tile([P, FC], BF16, tag="dna", name=f"dna{ch}")
            dn = sb.tile([P, FC], BF16, tag="dn", name=f"dn{ch}")
            lg = sb.tile([P, FC], F32, tag="lg", name=f"lg{ch}")
            rc = sb.tile([P, FC], BF16, tag="rc", name=f"rc{ch}")
            chunk_state[ch] = (xt, gxi, mk, tmp6, tmp3, dna, dn, lg, rc)

            # per-chunk loads: contiguous 110KB HBM block per (class, chunk)
            for c in range(C):
                eng = (nc.sync, nc.gpsimd, nc.scalar)[c % 3]
                eng.dma_start(xt[:, c], x_v[c, ch])
            nc.sync.dma_start(tt[:], t_v[ch])

            # interleaved copy of the raw logits for the PE x-block
            nc.vector.tensor_copy(
                gxi[:, :, C:].rearrange("p g c j -> p c g j"),
                xt[:].rearrange("p c (g j) -> p c g j", j=JB))

            # onehot masks, all classes in one op via broadcast operands
            tg_b = tt[:].rearrange("p (g j) -> p () g j", j=JB) \
                .to_broadcast([P, C, G, JB])
            cls_b = cls[:].rearrange("p c -> p c () ()") \
                .to_broadcast([P, C, G, JB])
            nc.vector.tensor_tensor(mk[:].rearrange("p g c j -> p c g j"),
                                    tg_b, cls_b, op=ALU.is_equal)

            # exp in-place, one op (after the xi snapshot; WAR ordered)
            nc.scalar.activation(xt[:], xt[:], ACTF.Exp)

            # softmax denominator: merged pairwise tree adds
            nc.vector.tensor_tensor(tmp6[:], xt[:, 0::2], xt[:, 1::2],
                                    op=ALU.add)
            nc.vector.tensor_tensor(tmp3[:], tmp6[:, 0::2], tmp6[:, 1::2],
                                    op=ALU.add)
            nc.vector.tensor_tensor(dna[:], tmp3[:, 0], tmp3[:, 1],
                                    op=ALU.add)
            nc.vector.tensor_tensor(dn[:], dna[:], tmp3[:, 2], op=ALU.add)

        def phase_ln(ch):
            (xt, gxi, mk, tmp6, tmp3, dna, dn, lg, rc) = chunk_state[ch]
            nc.scalar.activation(lg[:], dn[:], ACTF.Ln,
                                 accum_out=logdacc[:, ch:ch + 1])

        def phase_tail(ch):
            (xt, gxi, mk, tmp6, tmp3, dna, dn, lg, rc) = chunk_state[ch]
            nc.scalar.activation(rc[:], lg[:], ACTF.Exp, scale=-1.0)

            # probs g_c = exp_c * recip, one op via broadcast recip
            rc_b = rc[:].rearrange("p (g j) -> p () g j", j=JB) \
                .to_broadcast([P, C, G, JB])
            nc.vector.tensor_tensor(
                gxi[:, :, :C].rearrange("p g c j -> p c g j"),
                xt[:].rearrange("p c (g j) -> p c g j", j=JB),
                rc_b, op=ALU.mult)

            # statistics matrix on PE: one [128,96]x[128,192] matmul/group
            for g in range(G):
                nc.tensor.matmul(psum_m[:], mk[:, g], gxi[:, g],
                                 start=(ch == 0 and g == 0),
                                 stop=(ch == NCH - 1 and g == G - 1))

        for ch in range(NCH):
            phase_exp(ch)
            phase_ln(ch)
            phase_tail(ch)

        m_sb = acc.tile([12 * JB, 24 * JB], F32)
        nc.vector.tensor_copy(m_sb[:], psum_m[:])
        nc.sync.dma_start(m_d[:], m_sb[:])
        nc.sync.dma_start(logd_d[:], logdacc[:])

    nc.compile()
    return nc


def _get_program():
    if "nc" not in _CACHE:
        _CACHE["nc"] = _build_program()
    return _CACHE["nc"]


def _shard_inputs(input, target):
    """Full (2,12,96,96,96)/(2,1,96,96,96) -> 8 per-core in_maps (bf16)."""
    bf16 = mybir.dt.np(BF16)
    x = np.asarray(input, dtype=np.float32)
    tg = np.asarray(target).reshape(N, SPATIAL)
    in_maps = []
    for k in range(N_CORES):
        n = k // CORES_PER_N
        o = (k % CORES_PER_N) * S
        xs = np.ascontiguousarray(
            x[n].reshape(C, SPATIAL)[:, o:o + S]
            .reshape(C, P, NCH, FC).transpose(0, 2, 1, 3)
            .reshape(C, S)).astype(bf16)
        ts = np.ascontiguousarray(
            tg[n, o:o + S].reshape(P, NCH, FC).transpose(1, 0, 2)
            .reshape(S)).astype(np.float32).astype(bf16)
        in_maps.append({"x": xs, "t": ts,
                        "cls": np.arange(C, dtype=np.float32).astype(bf16)})
    return in_maps


def _combine(results, matrix_penalty, global_step, maxiter):
    pen = np.asarray(matrix_penalty, dtype=np.float64)
    inter = np.zeros((N, C))
    ground = np.zeros((N, C))
    pred = np.zeros((N, C))
    xtgt_sum = 0.0
    logd_sum = 0.0
    pen_sum = 0.0
    for k, r in enumerate(results):
        n = k // CORES_PER_N
        mfull = np.asarray(r["m_out"], dtype=np.float64) \
            .reshape(C, JB, 2 * C, JB)
        m = np.einsum("tjcj->tc", mfull)        # sum the JB diagonal blocks
        mg = m[:, :C]                           # sum_v m_t * g_c
        inter[n] += np.diag(mg)
        ground[n] += mg.sum(axis=1)
        pred[n] += mg.sum(axis=0)               # masks partition unity
        xtgt_sum += np.trace(m[:, C:2 * C])
        logd_sum += float(np.asarray(r["logd_out"], dtype=np.float64).sum())
        pen_sum += float((pen * mg).sum())

    nvox = N * SPATIAL
    dice = 1.0 - (2.0 * inter + 1e-5) / (ground + pred + 1e-5)
    dice_loss = dice.mean()
    ce = (logd_sum - xtgt_sum) / nvox
    ce_total = dice_loss + ce
    pen_mean = pen_sum / nvox
    beta = 10.0 ** np.floor(np.log10(ce_total))
    gs = float(global_step)
    mi = float(maxiter)
    alpha0 = 1.0 - gs / mi
    alpha1 = gs / mi
    return np.float32(alpha1 * ce_total + alpha0 * beta * pen_mean)


def kernel(input, target, matrix_penalty, global_step, maxiter):
    nc = _get_program()
    in_maps = _shard_inputs(input, target)
    trace = bool(int(os.environ.get("BASS_LOSS_TRACE", "0")))
    res = bass_utils.run_bass_kernel_spmd(
        nc, in_maps, core_ids=list(range(N_CORES)), trace=trace)
    _CACHE["last_exec_ns"] = res.exec_time_ns
    return _combine(res.results, matrix_penalty, global_step, maxiter)
